# revision 7
# baseline (speedup 1.0000x reference)
"""Trainium2 Bass kernel for the Neural3DMM spiral-conv mesh decoder.

Data-parallel: 2 batch elems per core x 8 cores.  Per core:
  fc -> up2 -> conv2(z2 = x @ W0stack) -> gather+reduce+elu -> up1 -> conv1
     -> gather+reduce+elu -> up0 (bf16) -> conv0 -> gather+diag-reduce -> out

Spiral convs use the commute trick: gather(x) @ W == sum_k gather_k(x @ W_k),
so the weight multiply happens densely BEFORE the gather and the gather+sum
runs over the output features.  Conv biases ride along as one extra gathered
row per tile.  Gathers are SWDGE dma_gather with host-prepared combined
(row, k-block) int16 indices.
"""

import sys

if '/opt/trn_rl_repo' not in sys.path:
    sys.path.insert(0, '/opt/trn_rl_repo')

import numpy as np
import ml_dtypes

import concourse.bacc as bacc
import concourse.tile as tile
import concourse.mybir as mybir
from concourse._compat import get_trn_type
from concourse.bass_types import AP
from concourse.bass_utils import run_bass_kernel_spmd

F32 = mybir.dt.float32
F32R = mybir.dt.float32r
BF16 = mybir.dt.bfloat16
I16 = mybir.dt.int16

SIZES = [11793, 2948, 737, 184]
V = [s + 1 for s in SIZES]          # [11794, 2949, 738, 185]
B = 16
NCORES = 8

NT0 = 93                              # ceil(11794/128)
NT1 = 24                              # ceil(2949/128), last tile 5 rows
NT2 = 6                               # ceil(738/128),  last tile 98 rows
Z0_ROWS = NT0 * 128                   # 11904 (incl. junk rows 11794..11903)
BIAS0_ROW = Z0_ROWS                   # 11904
BIAS1_SLOT = 7 * V[1]                 # 20643  (row 2949, k-slot 0 of z1a)
BIAS2_SLOT = 12 * V[2]                # 8856   (row 738, k-slot 0)


def _chunks(total, step):
    out = []
    o = 0
    while o < total:
        out.append((o, min(step, total - o)))
        o += step
    return out


def _r(ap):
    """matmul operand view (f32 for now; f32r needs producer-side rounding)"""
    return ap


def _elu(nc, pool, out_ap, pre):
    """out = elu(pre) = relu(pre) + (exp(min(pre,0)) - 1)"""
    n = pre.shape[-1]
    m = pool.tile([128, n], F32, tag="elu_m")
    nc.vector.tensor_scalar_min(m[:], pre[:], 0.0)
    e = pool.tile([128, n], F32, tag="elu_e")
    nc.scalar.activation(e[:], m[:], mybir.ActivationFunctionType.Exp)
    nc.vector.tensor_scalar_add(e[:], e[:], -1.0)
    r = pool.tile([128, n], F32, tag="elu_r")
    nc.vector.tensor_scalar_max(r[:], pre[:], 0.0)
    nc.vector.tensor_add(out_ap, e[:], r[:])


def _diag_reduce(nc, out_ap, gap, inner):
    """out[p, dims...] = sum_j g[p, <strided diag>]"""
    nc.vector.reduce_sum(
        out_ap,
        AP(tensor=gap.tensor, offset=gap.offset, ap=[gap.ap[0]] + inner),
        axis=mybir.AxisListType.X)


def build_kernel():
    nc = bacc.Bacc(get_trn_type() or "TRN2")

    zT = nc.dram_tensor("zT", [128, 2], F32, kind="ExternalInput")
    wfc = nc.dram_tensor("wfc", [128, 185 * 128], F32, kind="ExternalInput")
    bfcT = nc.dram_tensor("bfcT", [185, 128], F32, kind="ExternalInput")
    u2t = nc.dram_tensor("u2t", [185, 738], F32, kind="ExternalInput")
    u1t = nc.dram_tensor("u1t", [738, 2949], F32, kind="ExternalInput")
    u0t = nc.dram_tensor("u0t", [2949, Z0_ROWS], BF16, kind="ExternalInput")
    w0s = nc.dram_tensor("w0s", [128, 12 * 128], F32, kind="ExternalInput")
    w1s = nc.dram_tensor("w1s", [128, 14 * 64], F32, kind="ExternalInput")
    w2s = nc.dram_tensor("w2s", [128, 64], BF16, kind="ExternalInput")
    bias2x0 = nc.dram_tensor("bias2x0", [1, 12 * 256], F32, kind="ExternalInput")
    bias2x1 = nc.dram_tensor("bias2x1", [1, 7 * 128], F32, kind="ExternalInput")
    biaspat2 = nc.dram_tensor("biaspat2", [1, 128], F32, kind="ExternalInput")
    idx2 = nc.dram_tensor("idx2", [128, NT2 * 104], I16, kind="ExternalInput")
    idx1a = nc.dram_tensor("idx1a", [128, NT1 * 64], I16, kind="ExternalInput")
    idx1b = nc.dram_tensor("idx1b", [128, NT1 * 56], I16, kind="ExternalInput")
    idx0 = nc.dram_tensor("idx0", [128, NT0 * 168], I16, kind="ExternalInput")
    outv = nc.dram_tensor("outv", [Z0_ROWS, 6], F32, kind="ExternalOutput")

    fcd = nc.dram_tensor("fcd", [2, 185 * 128], F32)
    z2i = nc.dram_tensor("z2i", [V[2] + 1, 12 * 256], F32)      # [739, 3072]
    z1ia = nc.dram_tensor("z1ia", [V[1] + 1, 7 * 128], F32)     # [2950, 896]
    z1ib = nc.dram_tensor("z1ib", [V[1] + 1, 7 * 128], F32)
    z0i = nc.dram_tensor("z0i", [Z0_ROWS + 1, 128], F32)        # [11905, 128]

    with tile.TileContext(nc) as tc:
        # =============== bias rows of the gather tables ===============
        with tc.tile_pool(name="pbias", bufs=1) as pb:
            b0row = pb.tile([1, 12 * 256], F32)
            nc.sync.dma_start(b0row[:], bias2x0[:])
            nc.sync.dma_start(z2i[V[2]:V[2] + 1, :], b0row[:])
            b1row = pb.tile([1, 7 * 128], F32)
            nc.sync.dma_start(b1row[:], bias2x1[:])
            nc.sync.dma_start(z1ia[V[1]:V[1] + 1, :], b1row[:])
            zrow = pb.tile([1, 7 * 128], F32)
            nc.vector.memset(zrow[:], 0.0)
            nc.sync.dma_start(z1ib[V[1]:V[1] + 1, :], zrow[:])
            b2row = pb.tile([1, 128], F32)
            nc.sync.dma_start(b2row[:], biaspat2[:])
            nc.sync.dma_start(z0i[BIAS0_ROW:BIAS0_ROW + 1, :], b2row[:])

        with tc.tile_pool(name="pAB", bufs=1) as pab:
            x3a = pab.tile([128, 2, 128], F32)     # [v, b, f]
            x3b = pab.tile([57, 2, 128], F32)

            # =============== phase A: fc -> x3 ===============
            with tc.tile_pool(name="pA", bufs=1) as pa, \
                 tc.tile_pool(name="pAw", bufs=3) as paw, \
                 tc.tile_pool(name="psA", bufs=3, space="PSUM") as psa:
                zT_sb = pa.tile([128, 2], F32)
                nc.sync.dma_start(zT_sb[:], zT[:])
                fcflat = pa.tile([2, 185 * 128], F32)
                for (o, w) in _chunks(185 * 128, 512):
                    wfc_c = paw.tile([128, 512], F32, tag="wfc")
                    nc.sync.dma_start(wfc_c[:, :w], wfc[:, o:o + w])
                    ps = psa.tile([2, 512], F32, tag="psA")
                    nc.tensor.matmul(ps[:, :w], _r(zT_sb[:]), _r(wfc_c[:, :w]),
                                     start=True, stop=True)
                    nc.vector.tensor_copy(fcflat[:, o:o + w], ps[:, :w])
                nc.sync.dma_start(fcd[:], fcflat[:])
                # reshape via DRAM bounce + fc bias add
                fcv = fcd[:].rearrange("b (v f) -> v b f", f=128)
                nc.sync.dma_start(x3a[:], fcv[0:128])
                nc.sync.dma_start(x3b[:], fcv[128:185])
                bfa = pa.tile([128, 128], F32)
                bfb = pa.tile([57, 128], F32)
                nc.sync.dma_start(bfa[:], bfcT[0:128])
                nc.sync.dma_start(bfb[:], bfcT[128:185])
                for b in range(2):
                    nc.vector.tensor_add(x3a[:, b, :], x3a[:, b, :], bfa[:])
                    nc.vector.tensor_add(x3b[:, b, :], x3b[:, b, :], bfb[:])

            # =============== phase B: up2 + conv2 -> z2i ===============
            with tc.tile_pool(name="pB", bufs=1) as pbm, \
                 tc.tile_pool(name="pBw", bufs=2) as pbw, \
                 tc.tile_pool(name="psB", bufs=4, space="PSUM") as psb:
                u2a = pbm.tile([128, 738], F32)
                u2b = pbm.tile([57, 738], F32)
                nc.sync.dma_start(u2a[:], u2t[0:128])
                nc.sync.dma_start(u2b[:], u2t[128:185])
                x2upT = pbm.tile([128, 2, 738], F32)     # [f, b, v]
                for b in range(2):
                    for (o, w) in _chunks(738, 512):
                        ps = psb.tile([128, 512], F32, tag="psB")
                        nc.tensor.matmul(ps[:, :w], _r(x3a[:, b, :]),
                                         _r(u2a[:, o:o + w]),
                                         start=True, stop=False)
                        nc.tensor.matmul(ps[:, :w], _r(x3b[:, b, :]),
                                         _r(u2b[:, o:o + w]),
                                         start=False, stop=True)
                        nc.vector.tensor_copy(x2upT[:, b, o:o + w], ps[:, :w])

                w0s_sb = pbm.tile([128, 12 * 128], F32)
                nc.sync.dma_start(w0s_sb[:], w0s[:])
                for vt in range(NT2):
                    vw = min(128, 738 - vt * 128)
                    z2sb = pbw.tile([128, 12, 2, 128], F32, tag="z2sb")
                    for b in range(2):
                        for ci, (o, w) in enumerate(_chunks(12 * 128, 512)):
                            ps = psb.tile([128, 512], F32, tag="psB")
                            nc.tensor.matmul(
                                ps[:vw, :w],
                                _r(x2upT[:, b, vt * 128:vt * 128 + vw]),
                                _r(w0s_sb[:, o:o + w]), start=True, stop=True)
                            nc.vector.tensor_copy(
                                z2sb[:vw, 4 * ci:4 * ci + 4, b, :],
                                ps[:vw, :w].rearrange("p (k f) -> p k f", f=128))
                    nc.sync.dma_start(
                        z2i[vt * 128:vt * 128 + vw, :],
                        z2sb[:vw].rearrange("p a b f -> p (a b f)"))

        # =============== phase C: gather2+elu ; up1 ; conv1 -> z1 ===============
        with tc.tile_pool(name="pC", bufs=1) as pc, \
             tc.tile_pool(name="pCg", bufs=2) as pcg, \
             tc.tile_pool(name="pCw", bufs=2) as pcw, \
             tc.tile_pool(name="pCs", bufs=3) as pcs, \
             tc.tile_pool(name="psC", bufs=4, space="PSUM") as psc:
            idx2_sb = pc.tile([128, NT2 * 104], I16)
            nc.sync.dma_start(idx2_sb[:], idx2[:])
            z2g = z2i[:].rearrange("r (s e) -> (r s) e", e=256)
            x2f = pc.tile([128, NT2, 256], F32)
            for vt in range(NT2):
                g2 = pcg.tile([128, 13, 256], F32, tag="g2")
                nc.gpsimd.dma_gather(g2[:], z2g,
                                     idx2_sb[:, vt * 104:(vt + 1) * 104],
                                     13 * 128, 13 * 128, 256,
                                     single_packet=False)
                pre = pcs.tile([128, 256], F32, tag="pre")
                _diag_reduce(nc, pre[:], g2[:], [[1, 256], [256, 13]])
                _elu(nc, pcs, x2f[:, vt, :], pre)

            # up1: x1upT[f, b, v] accumulating over 6 k-tiles of x2f
            x1upT = pc.tile([128, 2, 2949], F32)
            for (o, w) in _chunks(2949, 512):
                u1c = pcw.tile([128, 6, 512], F32, tag="u1c")
                for kt in range(6):
                    kw = min(128, 738 - kt * 128)
                    nc.sync.dma_start(u1c[:kw, kt, :w],
                                      u1t[kt * 128:kt * 128 + kw, o:o + w])
                for b in range(2):
                    ps = psc.tile([128, 512], F32, tag="psC")
                    for kt in range(6):
                        kw = min(128, 738 - kt * 128)
                        nc.tensor.matmul(ps[:, :w],
                                         _r(x2f[:kw, kt, b * 128:(b + 1) * 128]),
                                         _r(u1c[:kw, kt, :w]),
                                         start=(kt == 0), stop=(kt == 5))
                    nc.vector.tensor_copy(x1upT[:, b, o:o + w], ps[:, :w])

            w1s_sb = pc.tile([128, 14 * 64], F32)
            nc.sync.dma_start(w1s_sb[:], w1s[:])
            for vt in range(NT1):
                vw = min(128, 2949 - vt * 128)
                z1sb = pcw.tile([128, 14, 2, 64], F32, tag="z1sb")
                for b in range(2):
                    for ci, (o, w) in enumerate(_chunks(14 * 64, 512)):
                        ps = psc.tile([128, 512], F32, tag="psC")
                        nc.tensor.matmul(
                            ps[:vw, :w],
                            _r(x1upT[:, b, vt * 128:vt * 128 + vw]),
                            _r(w1s_sb[:, o:o + w]), start=True, stop=True)
                        nc.vector.tensor_copy(
                            z1sb[:vw, 8 * ci:8 * ci + (w // 64), b, :],
                            ps[:vw, :w].rearrange("p (k f) -> p k f", f=64))
                nc.sync.dma_start(
                    z1ia[vt * 128:vt * 128 + vw, :],
                    z1sb[:vw, 0:7].rearrange("p a b f -> p (a b f)"))
                nc.sync.dma_start(
                    z1ib[vt * 128:vt * 128 + vw, :],
                    z1sb[:vw, 7:14].rearrange("p a b f -> p (a b f)"))

        # =============== phases D+E: gather1+elu -> x1f ; up0 ; conv0 ===============
        with tc.tile_pool(name="pDE", bufs=1) as pde:
            x1fb = pde.tile([128, NT1, 128], BF16)
            with tc.tile_pool(name="pD", bufs=1) as pd, \
                 tc.tile_pool(name="pDg", bufs=2) as pdg, \
                 tc.tile_pool(name="pDs", bufs=3) as pds:
                idx1a_sb = pd.tile([128, NT1 * 64], I16)
                idx1b_sb = pd.tile([128, NT1 * 56], I16)
                nc.sync.dma_start(idx1a_sb[:], idx1a[:])
                nc.sync.dma_start(idx1b_sb[:], idx1b[:])
                z1ga = z1ia[:].rearrange("r (s e) -> (r s) e", e=128)
                z1gb = z1ib[:].rearrange("r (s e) -> (r s) e", e=128)
                for vt in range(NT1):
                    g1 = pdg.tile([128, 15, 128], F32, tag="g1")
                    nc.gpsimd.dma_gather(g1[:, 0:8, :], z1ga,
                                         idx1a_sb[:, vt * 64:(vt + 1) * 64],
                                         8 * 128, 8 * 128, 128,
                                         single_packet=False)
                    nc.gpsimd.dma_gather(g1[:, 8:15, :], z1gb,
                                         idx1b_sb[:, vt * 56:(vt + 1) * 56],
                                         7 * 128, 7 * 128, 128,
                                         single_packet=False)
                    pre = pds.tile([128, 128], F32, tag="pre")
                    _diag_reduce(nc, pre[:], g1[:], [[1, 128], [128, 15]])
                    _elu(nc, pds, x1fb[:, vt, :], pre)

            with tc.tile_pool(name="pE", bufs=1) as pe, \
                 tc.tile_pool(name="pEw", bufs=2) as pew, \
                 tc.tile_pool(name="pEz", bufs=2) as pez, \
                 tc.tile_pool(name="psE", bufs=4, space="PSUM") as pse, \
                 tc.tile_pool(name="psE0", bufs=2, space="PSUM") as pse0:
                x0T = pe.tile([128, Z0_ROWS], BF16)      # [(b,f), v]
                kt0 = _chunks(2949, 128)
                for (o, w) in _chunks(Z0_ROWS, 512):
                    ps = pse.tile([128, 512], F32, tag="psE")
                    u0c = pew.tile([128, 24, 512], BF16, tag="u0c")
                    for kt, (ko, kw) in enumerate(kt0):
                        nc.sync.dma_start(u0c[:kw, kt, :w], u0t[ko:ko + kw, o:o + w])
                        nc.tensor.matmul(ps[:, :w], x1fb[:kw, kt, :],
                                         u0c[:kw, kt, :w],
                                         start=(kt == 0), stop=(kt == 23))
                    nc.vector.tensor_copy(x0T[:, o:o + w], ps[:, :w])

                w2s_sb = pe.tile([128, 64], BF16)
                nc.sync.dma_start(w2s_sb[:], w2s[:])
                for vt in range(NT0):
                    z0sb = pez.tile([128, 128], F32, tag="z0sb")
                    for b in range(2):
                        ps0 = pse0.tile([128, 64], F32, tag="ps0")
                        nc.tensor.matmul(ps0[:], x0T[b * 64:(b + 1) * 64,
                                                     vt * 128:(vt + 1) * 128],
                                         w2s_sb[b * 64:(b + 1) * 64, :],
                                         start=True, stop=True)
                        nc.vector.tensor_copy(z0sb[:, b * 64:(b + 1) * 64], ps0[:])
                    nc.sync.dma_start(z0i[vt * 128:(vt + 1) * 128, :], z0sb[:])

        # =============== phase F: gather0 + diag reduce -> outv ===============
        with tc.tile_pool(name="pF", bufs=1) as pf, \
             tc.tile_pool(name="pFg", bufs=3) as pfg, \
             tc.tile_pool(name="pFs", bufs=4) as pfs:
            idx0_sb = pf.tile([128, NT0 * 168], I16)
            nc.sync.dma_start(idx0_sb[:], idx0[:])
            for vt in range(NT0):
                g0 = pfg.tile([128, 21, 128], F32, tag="g0")
                nc.gpsimd.dma_gather(g0[:], z0i[:],
                                     idx0_sb[:, vt * 168:(vt + 1) * 168],
                                     21 * 128, 21 * 128, 128,
                                     single_packet=False)
                o6 = pfs.tile([128, 2, 3], F32, tag="o6")
                _diag_reduce(nc, o6[:], g0[:], [[64, 2], [1, 3], [131, 21]])
                nc.sync.dma_start(outv[vt * 128:(vt + 1) * 128, :],
                                  o6[:].rearrange("p a b -> p (a b)"))
    nc.compile()
    return nc


# ---------------------------------------------------------------------------
# host side
# ---------------------------------------------------------------------------

_NC_CACHE = None


def _get_nc():
    global _NC_CACHE
    if _NC_CACHE is None:
        _NC_CACHE = build_kernel()
    return _NC_CACHE


def _wrap_idx(flat):
    """flat int array -> dma_gather wrapped layout [128, n/16] int16"""
    w = np.ascontiguousarray(flat.reshape(-1, 16).T).astype(np.int16)
    return np.tile(w, (8, 1))


def _host_inputs(z, U0, U1, U2, S0, S1, S2, Wfc, bfc, W0, b0, W1, b1, W2, b2):
    f32 = np.float32
    common = {}
    common["wfc"] = np.ascontiguousarray(Wfc, f32)
    common["bfcT"] = np.ascontiguousarray(np.asarray(bfc, f32).reshape(185, 128))
    common["u2t"] = np.ascontiguousarray(np.asarray(U2, f32).T)
    u1t = np.ascontiguousarray(np.asarray(U1, f32).T)
    u1t[737, :] = 0.0        # dummy vertex of level 2 is zeroed pre-upsample
    common["u1t"] = u1t
    u0t = np.zeros((2949, Z0_ROWS), ml_dtypes.bfloat16)
    u0t[:, :V[0]] = np.asarray(U0, f32).T.astype(ml_dtypes.bfloat16)
    u0t[2948, :] = 0         # dummy vertex of level 1 is zeroed pre-upsample
    common["u0t"] = u0t
    common["w0s"] = np.ascontiguousarray(
        np.asarray(W0, f32).reshape(12, 128, 128).transpose(1, 0, 2).reshape(128, -1))
    common["w1s"] = np.ascontiguousarray(
        np.asarray(W1, f32).reshape(14, 128, 64).transpose(1, 0, 2).reshape(128, -1))
    w2s = np.zeros((64, 64), f32)
    w2s[:, :60] = np.asarray(W2, f32).reshape(20, 64, 3).transpose(1, 0, 2).reshape(64, 60)
    common["w2s"] = np.ascontiguousarray(np.vstack([w2s, w2s])).astype(ml_dtypes.bfloat16)
    b0w = np.zeros((1, 12 * 256), f32)
    b0w[0, :256] = np.tile(np.asarray(b0, f32), 2)
    common["bias2x0"] = b0w
    b1w = np.zeros((1, 7 * 128), f32)
    b1w[0, :128] = np.tile(np.asarray(b1, f32), 2)
    common["bias2x1"] = b1w
    bp2 = np.zeros((1, 128), f32)
    bp2[0, 60:63] = np.asarray(b2, f32)
    bp2[0, 124:127] = np.asarray(b2, f32)
    common["biaspat2"] = bp2

    S0 = np.asarray(S0).astype(np.int64)
    S1 = np.asarray(S1).astype(np.int64)
    S2 = np.asarray(S2).astype(np.int64)

    cols = []
    for vt in range(NT2):
        rows = np.clip(np.arange(vt * 128, vt * 128 + 128), 0, V[2] - 1)
        chunks = [S2[rows, j] * 12 + j for j in range(12)]
        chunks.append(np.full(128, BIAS2_SLOT, np.int64))
        cols.append(_wrap_idx(np.concatenate(chunks)))
    common["idx2"] = np.ascontiguousarray(np.concatenate(cols, axis=1))

    colsa, colsb = [], []
    for vt in range(NT1):
        rows = np.clip(np.arange(vt * 128, vt * 128 + 128), 0, V[1] - 1)
        ca = [S1[rows, j] * 7 + j for j in range(7)]
        ca.append(np.full(128, BIAS1_SLOT, np.int64))
        cb = [S1[rows, 7 + j] * 7 + j for j in range(7)]
        colsa.append(_wrap_idx(np.concatenate(ca)))
        colsb.append(_wrap_idx(np.concatenate(cb)))
    common["idx1a"] = np.ascontiguousarray(np.concatenate(colsa, axis=1))
    common["idx1b"] = np.ascontiguousarray(np.concatenate(colsb, axis=1))

    cols = []
    for vt in range(NT0):
        rows = np.clip(np.arange(vt * 128, vt * 128 + 128), 0, V[0] - 1)
        chunks = [S0[rows, j] for j in range(20)]
        chunks.append(np.full(128, BIAS0_ROW, np.int64))
        cols.append(_wrap_idx(np.concatenate(chunks)))
    common["idx0"] = np.ascontiguousarray(np.concatenate(cols, axis=1))

    z = np.asarray(z, f32)
    in_maps = []
    for c in range(NCORES):
        m = dict(common)
        m["zT"] = np.ascontiguousarray(z[2 * c:2 * c + 2].T)
        in_maps.append(m)
    return in_maps


def run(trace=False, **inputs):
    nc = _get_nc()
    in_maps = _host_inputs(**{k: np.asarray(v) for k, v in inputs.items()})
    res = run_bass_kernel_spmd(nc, in_maps, core_ids=list(range(NCORES)),
                               trace=trace)
    out = np.empty((B, V[0], 3), np.float32)
    for c in range(NCORES):
        ov = res.results[c]["outv"]            # [11904, 6]
        for j in range(2):
            out[2 * c + j] = ov[:V[0], 3 * j:3 * j + 3]
    out[:, V[0] - 1, :] = 0.0          # dummy vertex of level 0
    return out, res


def kernel(**inputs):
    out, _ = run(**inputs)
    return out
